# revision 1
# baseline (speedup 1.0000x reference)
"""Trainium2 Bass kernel for nn_DigitConvolutionalModel (dense CNN -> MLP).

Strategy: pure data parallel over 8 NeuronCores (2048 samples each).
The 3x3 conv (VALID, single channel) is linear, so it is folded into the
first FC layer on the host: W1e[784, 256] = C @ w1.T where C is the
sparse conv operator. The whole network then becomes a 4-layer MLP:

    out = relu(relu(relu(x @ W1e + b1) @ w2.T + b2) @ w3.T + b3) @ w4.T + b4

On-device everything is computed in "transposed" orientation (features on
partitions, batch on the free dim), so the only transpose needed is of x,
done on the host. Matmuls run as float32r (fp22 multiply, fp32 accumulate)
which is full PE speed at N>=512 with ~1e-4 relative error.
"""

import numpy as np

import concourse.bacc as bacc
import concourse.mybir as mybir
import concourse.tile as tile
from concourse import bass_utils

N_CORES = 8
B = 16384
BC = B // N_CORES  # 2048 samples per core
NB = 512  # batch tile (free dim), one PSUM bank of fp32
NT = BC // NB  # 4 batch tiles per core
KC = 112  # K chunk for the 784-deep first layer
NKC = 784 // KC  # 7 chunks

F32 = mybir.dt.float32
F32R = mybir.dt.float32r
RELU = mybir.ActivationFunctionType.Relu
IDENT = mybir.ActivationFunctionType.Identity

_PROGRAM = None


def _build_program():
    nc = bacc.Bacc(None)

    xT_d = nc.declare_dram_parameter("xT", [784, BC], F32R, isOutput=False)
    w1_d = nc.declare_dram_parameter("w1e", [784, 256], F32R, isOutput=False)
    w2_d = nc.declare_dram_parameter("w2t", [256, 128], F32R, isOutput=False)
    w3_d = nc.declare_dram_parameter("w3t", [128, 64], F32R, isOutput=False)
    w4_d = nc.declare_dram_parameter("w4t", [64, 10], F32R, isOutput=False)
    b1_d = nc.declare_dram_parameter("b1", [128, 2], F32, isOutput=False)
    b2_d = nc.declare_dram_parameter("b2", [128, 1], F32, isOutput=False)
    b3_d = nc.declare_dram_parameter("b3", [64, 1], F32, isOutput=False)
    b4_d = nc.declare_dram_parameter("b4", [10, 1], F32, isOutput=False)
    out_d = nc.declare_dram_parameter("outT", [10, BC], F32, isOutput=True)

    with tile.TileContext(nc) as tc:
        with (
            tc.tile_pool(name="weights", bufs=1) as wpool,
            tc.tile_pool(name="xin", bufs=3) as xpool,
            tc.tile_pool(name="acts", bufs=2) as apool,
            tc.tile_pool(name="ps1", bufs=2, space="PSUM") as ps1pool,
            tc.tile_pool(name="ps2", bufs=2, space="PSUM") as ps2pool,
            tc.tile_pool(name="ps34", bufs=2, space="PSUM") as ps34pool,
        ):
            # Weights, loaded once. K chunks packed along the free dim.
            w1sb = wpool.tile([KC, NKC, 256], F32R, tag="w1")
            nc.sync.dma_start(
                w1sb[:], w1_d[:].rearrange("(c p) o -> p c o", p=KC)
            )
            w2sb = wpool.tile([128, 2, 128], F32R, tag="w2")
            nc.sync.dma_start(
                w2sb[:], w2_d[:].rearrange("(c p) o -> p c o", p=128)
            )
            w3sb = wpool.tile([128, 64], F32R, tag="w3")
            nc.sync.dma_start(w3sb[:], w3_d[:])
            w4sb = wpool.tile([64, 10], F32R, tag="w4")
            nc.sync.dma_start(w4sb[:], w4_d[:])
            b1sb = wpool.tile([128, 2], F32, tag="b1")
            nc.sync.dma_start(b1sb[:], b1_d[:])
            b2sb = wpool.tile([128, 1], F32, tag="b2")
            nc.sync.dma_start(b2sb[:], b2_d[:])
            b3sb = wpool.tile([64, 1], F32, tag="b3")
            nc.sync.dma_start(b3sb[:], b3_d[:])
            b4sb = wpool.tile([10, 1], F32, tag="b4")
            nc.sync.dma_start(b4sb[:], b4_d[:])

            for t in range(NT):
                # x^T tile [112, 7, 512]; two DMAs to spread across queues
                xsb = xpool.tile([KC, NKC, NB], F32R, tag="x")
                xview = xT_d[:, t * NB : (t + 1) * NB].rearrange(
                    "(c p) b -> p c b", p=KC
                )
                nc.sync.dma_start(xsb[:, 0:4, :], xview[:, 0:4, :])
                nc.sync.dma_start(xsb[:, 4:NKC, :], xview[:, 4:NKC, :])

                # Layer 1: h1 = relu(W1e.T @ x) [256, 512]
                ps1 = ps1pool.tile([128, 2, NB], F32, tag="ps1")
                for m in range(2):
                    for c in range(NKC):
                        nc.tensor.matmul(
                            ps1[:, m, :],
                            w1sb[:, c, m * 128 : (m + 1) * 128],
                            xsb[:, c, :],
                            start=(c == 0),
                            stop=(c == NKC - 1),
                        )
                h1sb = apool.tile([128, 2, NB], F32R, tag="h1")
                for m in range(2):
                    nc.scalar.activation(
                        h1sb[:, m, :], ps1[:, m, :], RELU, bias=b1sb[:, m : m + 1]
                    )

                # Layer 2: h2 = relu(w2.T... ) [128, 512]
                ps2 = ps2pool.tile([128, NB], F32, tag="ps2")
                for c in range(2):
                    nc.tensor.matmul(
                        ps2[:],
                        w2sb[:, c, :],
                        h1sb[:, c, :],
                        start=(c == 0),
                        stop=(c == 1),
                    )
                h2sb = apool.tile([128, NB], F32R, tag="h2")
                nc.scalar.activation(h2sb[:], ps2[:], RELU, bias=b2sb[:])

                # Layer 3: h3 [64, 512]
                ps3 = ps34pool.tile([64, NB], F32, tag="ps34")
                nc.tensor.matmul(ps3[:], w3sb[:], h2sb[:], start=True, stop=True)
                h3sb = apool.tile([64, NB], F32R, tag="h3")
                nc.scalar.activation(h3sb[:], ps3[:], RELU, bias=b3sb[:])

                # Layer 4: outT [10, 512] (no relu)
                ps4 = ps34pool.tile([10, NB], F32, tag="ps34")
                nc.tensor.matmul(ps4[:], w4sb[:], h3sb[:], start=True, stop=True)
                osb = apool.tile([10, NB], F32, tag="o")
                nc.scalar.activation(osb[:], ps4[:], IDENT, bias=b4sb[:])
                nc.sync.dma_start(out_d[:, t * NB : (t + 1) * NB], osb[:])

    nc.compile()
    return nc


def _get_program():
    global _PROGRAM
    if _PROGRAM is None:
        _PROGRAM = _build_program()
    return _PROGRAM


def _fold_conv_into_w1(conv_w, w1):
    """W1e[784, 256] such that x @ W1e == conv(x, conv_w).flat @ w1.T."""
    w1v = np.ascontiguousarray(w1.T).reshape(26, 26, 256)
    w1e = np.zeros((28, 28, 256), dtype=np.float32)
    for di in range(3):
        for dj in range(3):
            w1e[di : di + 26, dj : dj + 26, :] += conv_w[di, dj] * w1v
    return w1e.reshape(784, 256)


def kernel(x, conv_w, w1, b1, w2, b2, w3, b3, w4, b4):
    x = np.asarray(x, dtype=np.float32)
    conv_w = np.asarray(conv_w, dtype=np.float32)
    w1 = np.asarray(w1, dtype=np.float32)
    b1 = np.asarray(b1, dtype=np.float32)
    w2 = np.asarray(w2, dtype=np.float32)
    b2 = np.asarray(b2, dtype=np.float32)
    w3 = np.asarray(w3, dtype=np.float32)
    b3 = np.asarray(b3, dtype=np.float32)
    w4 = np.asarray(w4, dtype=np.float32)
    b4 = np.asarray(b4, dtype=np.float32)

    nc = _get_program()

    w1e = _fold_conv_into_w1(conv_w, w1)
    shared = {
        "w1e": w1e,
        "w2t": np.ascontiguousarray(w2.T),
        "w3t": np.ascontiguousarray(w3.T),
        "w4t": np.ascontiguousarray(w4.T),
        "b1": np.ascontiguousarray(b1.reshape(2, 128).T),
        "b2": b2.reshape(128, 1),
        "b3": b3.reshape(64, 1),
        "b4": b4.reshape(10, 1),
    }
    in_maps = []
    for m in range(N_CORES):
        xT = np.ascontiguousarray(x[m * BC : (m + 1) * BC].T)
        in_maps.append({"xT": xT, **shared})

    res = bass_utils.run_bass_kernel_spmd(nc, in_maps, list(range(N_CORES)))
    out = np.concatenate(
        [np.ascontiguousarray(res.results[m]["outT"].T) for m in range(N_CORES)],
        axis=0,
    )
    return out.astype(np.float32)
